# revision 22
# baseline (speedup 1.0000x reference)
"""Trainium2 Bass kernel for CusMultiHeadAttention.

Shapes (hardcoded): x (4,1024,1024) f32, bias (4,16,1024,1024) f32,
attention_mask (4,1024) i32, Wq/Wk/Wv (1024,1024), Wo (1024,1024), bo (1024,).

Sharding: 8 cores = 4 batches x 2 head-groups (8 heads each).
Wq/Wk/Wv column-parallel, Wo row-parallel (host sums the pair partials + bo).

Key-axis compaction: masked keys contribute exactly zero attention weight,
so the host gathers only the valid key positions (padded to a multiple of
128, slack slots killed by the same mask-multiply on V). The k-dim of the
K/V projections, scores, AV and the bias DMA all shrink by ~S_K/S.

Per-core pipeline (all "transposed" orientation, no on-device transposes):
  xT = x[b].T, xkT = x[b][valid].T (host)  -> SBUF (c_in on partitions)
  kT = Wk'^T @ xkT                         (feature on partitions, k free)
  v  = xk[b] @ Wv'                         (k on partitions, feature free)
  v_aug[h] = [v[h] * mask | mask]          (mask folded into V + ones-col)
  qT = (Wq'/8)^T @ xT                      (feature on partitions, q free)
  sT[h,kt] = kT[h,kt].T @ qT[h] + biasT    (k on partitions, q free)
  pT = exp(sT)                             (no max subtraction; scores O(5))
  o_aug[h] = sum_kt v_aug[h,kt].T @ pT[h,kt]
    v_aug uses a 64-wide ones BLOCK (matmul cost is N-driven, so M=128 is
    free): even head = [v | 1*64] -> o at partitions 0:64, denom replicated
    at 64:128; odd head = [1*64 | v] -> denom at 0:64, o at 64:128. One
    vector reciprocal + one small SBUF shift DMA aligns denom with o, and
    both heads write their half of the oT pair in-lane.
  outp = sum_pairs oT[pair].T @ Wo'[pair]  (K=128 contraction) -> DRAM
"""

import sys

if "/opt/trn_rl_repo" not in sys.path:
    sys.path.insert(0, "/opt/trn_rl_repo")

import math
from contextlib import ExitStack

import numpy as np

import concourse.mybir as mybir
import concourse.tile as tile
from concourse import bacc
from concourse.alu_op_type import AluOpType
from concourse.bass_utils import run_bass_kernel_spmd

B, S, C_IN = 4, 1024, 1024
N_HEAD, C = 16, 64
N_CORES = 8
HG = 8  # heads per core
NP = HG // 2  # head pairs per core
F = HG * C  # 512 local features
P = 128
KT = C_IN // P  # 8 contraction tiles for projections
ST = S // P  # 8 seq tiles (q side, always full)
VW = C + 1  # 65: v columns + ones-column

f32 = mybir.dt.float32
bf16 = mybir.dt.bfloat16


def build_program(s_k, taps=False):
    """s_k: padded key-slot count (multiple of 128)."""
    stk = s_k // P  # key tiles
    nc = bacc.Bacc("TRN2", target_bir_lowering=False, debug=False,
                   num_devices=N_CORES)
    if taps:
        dbg_v = nc.dram_tensor("dbg_v", (P, stk, NP, 2, 2 * C), bf16,
                               kind="ExternalOutput").ap()
        dbg_kT = nc.dram_tensor("dbg_kT", (P, F // P, s_k), bf16,
                                kind="ExternalOutput").ap()
        dbg_qT = nc.dram_tensor("dbg_qT", (P, F // P, S), bf16,
                                kind="ExternalOutput").ap()
        dbg_oT = nc.dram_tensor("dbg_oT", (P, NP, S), bf16,
                                kind="ExternalOutput").ap()
        dbg_oaug = nc.dram_tensor("dbg_oaug", (2, P, S), f32,
                                  kind="ExternalOutput").ap()
        dbg_rcb = nc.dram_tensor("dbg_rcb", (2, P, S), f32,
                                 kind="ExternalOutput").ap()

    xT = nc.dram_tensor("xT", (C_IN, S), bf16, kind="ExternalInput").ap()
    xkT = nc.dram_tensor("xkT", (C_IN, s_k), bf16, kind="ExternalInput").ap()
    wq = nc.dram_tensor("wq", (C_IN, F), bf16, kind="ExternalInput").ap()
    wk = nc.dram_tensor("wk", (C_IN, F), bf16, kind="ExternalInput").ap()
    wv = nc.dram_tensor("wv", (C_IN, F), bf16, kind="ExternalInput").ap()
    wo = nc.dram_tensor("wo", (F, C_IN), bf16, kind="ExternalInput").ap()
    biasT = nc.dram_tensor("biasT", (HG, s_k, S), bf16,
                           kind="ExternalInput").ap()
    maskf = nc.dram_tensor("maskf", (s_k,), f32, kind="ExternalInput").ap()
    outp = nc.dram_tensor("outp", (S, C_IN), bf16,
                          kind="ExternalOutput").ap()

    def kchunks():
        # split the s_k free dim into PSUM-bank-sized (<=512) chunks
        c0 = 0
        while c0 < s_k:
            cw = min(512, s_k - c0)
            yield c0, cw
            c0 += cw

    with tile.TileContext(nc) as tc:
        with ExitStack() as ctx:
            persist = ctx.enter_context(tc.tile_pool(name="persist", bufs=1))
            mask_sb = persist.tile([P, stk], f32)
            nc.sync.dma_start(mask_sb[:], maskf.rearrange("(t p) -> p t", p=P))
            ones_sb = persist.tile([P, NP, C], f32)
            nc.vector.memset(ones_sb[:], 1.0)
            # per (key-tile, pair, head-in-pair): [v | 1*64] or [1*64 | v]
            v_sb = persist.tile([P, stk, NP, 2, 2 * C], bf16)
            qT_sb = persist.tile([P, F // P, S], bf16)
            kT_sb = persist.tile([P, F // P, s_k], bf16)
            # head-pair rows: even head at partitions 0..63, odd at 64..127
            wo_sb = persist.tile([P, NP, C_IN], bf16)
            oT_sb = persist.tile([P, NP, S], bf16)

            # ---- phase A: projections (K/V first, then Q) ----
            with tc.tile_pool(name="phaseA", bufs=1) as pa, \
                 tc.tile_pool(name="psA", bufs=4, space="PSUM") as psA:
                xkT_sb = pa.tile([P, KT, s_k], bf16)
                wk_sb = pa.tile([P, KT, F], bf16)
                wv_sb = pa.tile([P, KT, F], bf16)
                xT_sb = pa.tile([P, KT, S], bf16)
                wq_sb = pa.tile([P, KT, F], bf16)
                for kt in range(KT):
                    nc.sync.dma_start(xkT_sb[:, kt, :],
                                      xkT[kt * P:(kt + 1) * P, :])
                    nc.sync.dma_start(wk_sb[:, kt, :],
                                      wk[kt * P:(kt + 1) * P, :])
                    nc.sync.dma_start(wv_sb[:, kt, :],
                                      wv[kt * P:(kt + 1) * P, :])
                for kt in range(KT):
                    nc.sync.dma_start(xT_sb[:, kt, :],
                                      xT[kt * P:(kt + 1) * P, :])
                    nc.sync.dma_start(wq_sb[:, kt, :],
                                      wq[kt * P:(kt + 1) * P, :])
                # wo is not needed until the output projection: issue its
                # DMA after the phase-A inputs on the in-order queue
                nc.sync.dma_start(
                    wo_sb[:], wo.rearrange("(p f) n -> f p n", f=P))

                # K / V projection groups. kT: (feature on partitions, k
                # free); v natural (k on partitions), mask+ones folded.
                def k_mm(mt, ps, kt):
                    for c0, cw in kchunks():
                        nc.tensor.matmul(
                            ps[:, c0:c0 + cw],
                            wk_sb[:, kt, mt * P:(mt + 1) * P],
                            xkT_sb[:, kt, c0:c0 + cw],
                            start=(kt == 0), stop=(kt == KT - 1))

                def v_mm(mt, ps, kt):
                    nc.tensor.matmul(
                        ps[:, 0:F],
                        xkT_sb[:, kt, mt * P:(mt + 1) * P],
                        wv_sb[:, kt, :],
                        start=(kt == 0), stop=(kt == KT - 1))

                def k_consume(mt, ps):
                    nc.scalar.copy(kT_sb[:, mt, :], ps[:, 0:s_k])

                def v_consume(mt, ps):
                    m_col = mask_sb[:, mt:mt + 1]
                    pv = ps[:, 0:F].rearrange("p (r t c) -> p r t c",
                                              t=2, c=C)
                    vv = v_sb[:, mt, :, :, :]
                    nc.vector.tensor_scalar_mul(
                        vv[:, :, 0, 0:C], pv[:, :, 0, :], m_col)
                    nc.vector.tensor_scalar_mul(
                        vv[:, :, 1, C:2 * C], pv[:, :, 1, :], m_col)
                    nc.vector.tensor_scalar_mul(
                        vv[:, :, 0, C:2 * C], ones_sb[:], m_col)
                    nc.vector.tensor_scalar_mul(
                        vv[:, :, 1, 0:C], ones_sb[:], m_col)

                # WAVEFRONT = True interleaves the first K/V groups' kt
                # loops so the in-order tensor engine tracks DMA arrival.
                WAVEFRONT = True
                if WAVEFRONT:
                    wave = [(k_mm, k_consume, 0), (k_mm, k_consume, 1)]
                    wave += [(v_mm, v_consume, m) for m in range(min(2, stk))]
                    tiles = {i: psA.tile([P, S], f32, name="ps_a")
                             for i in range(len(wave))}
                    for kt in range(KT):
                        for i, (mm, _, mt) in enumerate(wave):
                            mm(mt, tiles[i], kt)
                    for i, (_, consume, mt) in enumerate(wave):
                        consume(mt, tiles[i])
                k_rest = range(2, F // P) if WAVEFRONT else range(F // P)
                v_rest = range(2, stk) if WAVEFRONT else range(stk)
                for mt in k_rest:
                    ps = psA.tile([P, S], f32, name="ps_a")
                    for kt in range(KT):
                        k_mm(mt, ps, kt)
                    k_consume(mt, ps)
                for mt in v_rest:
                    ps = psA.tile([P, S], f32, name="ps_a")
                    for kt in range(KT):
                        v_mm(mt, ps, kt)
                    v_consume(mt, ps)

                # qT: (feature on partitions, q free)
                for mt in range(F // P):
                    ps = psA.tile([P, S], f32, name="ps_a")
                    for nh in range(2):
                        for kt in range(KT):
                            nc.tensor.matmul(
                                ps[:, nh * 512:(nh + 1) * 512],
                                wq_sb[:, kt, mt * P:(mt + 1) * P],
                                xT_sb[:, kt, nh * 512:(nh + 1) * 512],
                                start=(kt == 0), stop=(kt == KT - 1))
                    nc.scalar.copy(qT_sb[:, mt, :], ps[:])

            # ---- phase B: attention ----
            with tc.tile_pool(name="bias", bufs=8) as bias_pool, \
                 tc.tile_pool(name="pT", bufs=6) as pT_pool, \
                 tc.tile_pool(name="rc", bufs=2) as rc_pool, \
                 tc.tile_pool(name="rcb", bufs=2) as rcb_pool, \
                 tc.tile_pool(name="psS", bufs=4, space="PSUM") as psS, \
                 tc.tile_pool(name="psO", bufs=2, space="PSUM") as psO:

                # Software-pipelined over (h, kt, nh) half-tiles. The bias
                # is DMA'd bf16 and vector-copied into PSUM BEFORE the
                # scores matmul accumulates onto it (start=False), so the
                # bias add is off the tensor critical path; the AV matmul
                # of each item is emitted one item late so exp(i) overlaps
                # scores(i+1) on the in-order tensor queue.
                oaps_by_h = {}
                AV_DELAY = 3  # items the AV matmul lags the scores matmul
                pending = []  # of (h, nh, kt, pt_half)

                def emit_av(p):
                    ph, pnh, pkt, ppt = p
                    ppr, ptt = ph // 2, ph % 2
                    nc.tensor.matmul(
                        oaps_by_h[ph][:, pnh * 512:(pnh + 1) * 512],
                        v_sb[:, pkt, ppr, ptt, :],
                        ppt[:],
                        start=(pkt == 0), stop=(pkt == stk - 1))

                def finish_head(h):
                    oaps = oaps_by_h.pop(h)
                    pr, t = h // 2, h % 2
                    rc = rc_pool.tile([P, S], f32, name="rc")
                    rcb = rcb_pool.tile([P, S], f32, name="rcb")
                    if t == 0:
                        # reciprocal_approx_fast (custom DVE) is broken at
                        # base partition 64: copy denom down first, recip
                        # at base 0.
                        nc.scalar.copy(rc[C:P, :], oaps[C:P, :])
                        rcs = rc_pool.tile([P, S], f32, name="rcs",
                                           tag="rcs")
                        nc.sync.dma_start(rcs[0:C, :], rc[C:P, :])
                        nc.vector.reciprocal_approx_fast(rcb[0:C, :],
                                                         rcs[0:C, :])
                        nc.vector.tensor_mul(oT_sb[0:C, pr, :],
                                             oaps[0:C, :], rcb[0:C, :])
                    else:
                        nc.vector.reciprocal_approx_fast(rc[0:C, :],
                                                         oaps[0:C, :])
                        nc.sync.dma_start(rcb[C:P, :], rc[0:C, :])
                        nc.vector.tensor_mul(oT_sb[C:P, pr, :],
                                             oaps[C:P, :], rcb[C:P, :])

                for h in range(HG):
                    po = (h % 2) * C  # partition offset of head in qT/kT
                    pr = h // 2
                    kT_h = kT_sb[po:po + C, pr, :]
                    qT_h = qT_sb[po:po + C, pr, :]
                    oaps_by_h[h] = psO.tile([P, S], f32, name="oaug")
                    for kt in range(stk):
                        for nh in range(2):
                            ps = psS.tile([P, 512], f32, name="ps_s")
                            bt = bias_pool.tile([P, 512], bf16, name="bt")
                            nc.sync.dma_start(
                                bt[:], biasT[h, kt * P:(kt + 1) * P,
                                             nh * 512:(nh + 1) * 512])
                            nc.vector.tensor_scalar_mul(ps[:], bt[:], 1.0)
                            nc.tensor.matmul(
                                ps[:],
                                kT_h[:, kt * P:(kt + 1) * P],
                                qT_h[:, nh * 512:(nh + 1) * 512],
                                start=False, stop=True,
                                skip_group_check=True)
                            pt = pT_pool.tile([P, 512], bf16, name="pt")
                            nc.scalar.activation(
                                pt[:], ps[:],
                                mybir.ActivationFunctionType.Exp)
                            pending.append((h, nh, kt, pt))
                            if len(pending) > AV_DELAY:
                                p = pending.pop(0)
                                emit_av(p)
                                if p[2] == stk - 1 and p[1] == 1:
                                    finish_head(p[0])
                for p in pending:
                    emit_av(p)
                    if p[2] == stk - 1 and p[1] == 1:
                        finish_head(p[0])

                if taps:
                    nc.sync.dma_start(dbg_v, v_sb[:])
                    nc.sync.dma_start(dbg_kT, kT_sb[:])
                    nc.sync.dma_start(dbg_qT, qT_sb[:])
                    nc.sync.dma_start(dbg_oT, oT_sb[:])

            # ---- output projection (row-parallel partial, K=128 pairs) ----
            with tc.tile_pool(name="outsb", bufs=4) as out_pool, \
                 tc.tile_pool(name="psOut", bufs=3, space="PSUM") as psOut:
                for qt in range(ST):
                    for nh in range(2):
                        pso = psOut.tile([P, 512], f32, name="pso")
                        for p in range(NP):
                            nc.tensor.matmul(
                                pso[:],
                                oT_sb[:, p, qt * P:(qt + 1) * P],
                                wo_sb[:, p, nh * 512:(nh + 1) * 512],
                                start=(p == 0), stop=(p == NP - 1))
                        osb = out_pool.tile([P, 512], bf16, name="osb")
                        nc.scalar.copy(osb[:], pso[:])
                        nc.sync.dma_start(
                            outp[qt * P:(qt + 1) * P,
                                 nh * 512:(nh + 1) * 512],
                            osb[:])

    nc.compile()
    return nc


def make_in_maps(x, bias, attention_mask, Wq, Wk, Wv, Wo):
    import ml_dtypes
    bf = ml_dtypes.bfloat16
    scale = 1.0 / math.sqrt(C)
    wq_scaled = (np.asarray(Wq) * scale).astype(bf)
    x = np.asarray(x)
    bias = np.asarray(bias)
    mask = np.asarray(attention_mask)
    wk16 = np.asarray(Wk).astype(bf)
    wv16 = np.asarray(Wv).astype(bf)
    wo16 = np.asarray(Wo).astype(bf)

    idxs = [np.nonzero(mask[b])[0] for b in range(B)]
    nmax = max((len(i) for i in idxs), default=1)
    s_k = max(P, ((int(nmax) + P - 1) // P) * P)

    in_maps = []
    for c in range(N_CORES):
        b, hg = c // 2, c % 2
        fs = slice(hg * F, (hg + 1) * F)
        idx = idxs[b]
        nv = len(idx)
        xk = np.zeros((s_k, C_IN), dtype=np.float32)
        xk[:nv] = x[b][idx]
        maskc = np.zeros((s_k,), dtype=np.float32)
        maskc[:nv] = 1.0
        # biasT[h, k, q] = bias[b, head, q, k_valid]
        bT = np.zeros((HG, s_k, S), dtype=bf)
        bsel = bias[b, hg * HG:(hg + 1) * HG][:, :, idx]  # (HG, q, nv)
        bT[:, :nv, :] = bsel.transpose(0, 2, 1).astype(bf)
        in_maps.append({
            "xT": np.ascontiguousarray(x[b].T.astype(bf)),
            "xkT": np.ascontiguousarray(xk.T.astype(bf)),
            "wq": np.ascontiguousarray(wq_scaled[:, fs]),
            "wk": np.ascontiguousarray(wk16[:, fs]),
            "wv": np.ascontiguousarray(wv16[:, fs]),
            "wo": np.ascontiguousarray(wo16[fs, :]),
            "biasT": np.ascontiguousarray(bT),
            "maskf": maskc,
        })
    return in_maps, s_k


_NC_CACHE = {}


def get_program(s_k=640):
    if s_k not in _NC_CACHE:
        _NC_CACHE[s_k] = build_program(s_k)
    return _NC_CACHE[s_k]


def run(in_maps, s_k=640, trace=False, **kw):
    nc = get_program(s_k)
    return run_bass_kernel_spmd(nc, in_maps, core_ids=list(range(N_CORES)),
                                trace=trace, **kw)


def kernel(x, bias, attention_mask, Wq, Wk, Wv, Wo, bo):
    in_maps, s_k = make_in_maps(x, bias, attention_mask, Wq, Wk, Wv, Wo)
    res = run(in_maps, s_k=s_k)
    out = np.empty((B, S, C_IN), dtype=np.float32)
    for b in range(B):
        out[b] = (np.asarray(res.results[2 * b]["outp"], dtype=np.float32)
                  + np.asarray(res.results[2 * b + 1]["outp"],
                               dtype=np.float32)
                  + np.asarray(bo).astype(np.float32))
    return out
